# revision 46
# baseline (speedup 1.0000x reference)
"""Trainium2 Bass kernel for the CoOccurrenceEncoder pairwise-MLP problem.

Reference computation (per batch b of 4, N=512 nodes, d=128):
    hi = x @ W1[:d]          # [N, d]
    hj = x @ W1[d:]          # [N, d]
    h  = relu(hi[:,None,:] + hj[None,:,:] + b1)       # [N, N, d]
    h2 = relu(h @ W2 + b2)                            # [N, N, 64]
    out = sigmoid((h2 @ W3 + b3)[..., 0])             # [N, N]

Sharding: 8 cores; core c handles batch c//2, i-rows [256*(c%2), 256*(c%2)+256).
Each core holds its batch's full hj [d=128 partitions, N=512] in SBUF (bf16) and
streams 256 i-rows; weights are tiny and replicated.

Per-core dataflow (d=128 on partitions everywhere):
  stage1 (DVE, bf16 2x_1P): S_i = relu(hj + (hi_i + b1)) via one dual-op
          tensor_scalar (add per-partition vector, max 0) per row -> SBUF bf16
  stage2 (PE): stationary [W2 | W2] (128x128 bf16); a row PAIR runs as two
          column-tiled matmuls that co-start -> h2 fp32 [128, 2*512] PSUM
  stage2b (ACT): relu(h2 + b2) PSUM -> SBUF bf16 (1x, PSUM-source bound)
  stage3 (PE, software-pipelined LAG iterations behind stage2): 64 pairs
          accumulate densely into ONE psum bank; pair slot k uses the
          128-wide stationary window zbig[:, 126-2k:254-2k] whose W3 pair
          sits at array cols 2k,2k+1 (all other cols zero -> accumulate +0),
          so logits for 128 consecutive i-rows land on partitions 0..127.
  stage4 (ACT): one sigmoid(logits + b3) [128,512] per 64 pairs -> one
          contiguous [128,512] HWDGE DMA to HBM.
"""

import numpy as np
import ml_dtypes

import concourse.bass as bass
import concourse.mybir as mybir
import concourse.tile as tile
from concourse import bacc
from concourse.bass_utils import run_bass_kernel_spmd

F32 = mybir.dt.float32
BF16 = mybir.dt.bfloat16

D = 128          # feature dim (= partitions)
N = 512          # nodes per batch
B = 4            # batches
NCORES = 8
ROWS = 256       # i-rows per core
PAIRS = ROWS // 2
NQ = PAIRS // 2  # loop iterations, 2 pairs (4 rows) each

# every DVE_RELU_PERIOD-th q does its stage2-relu on DVE instead of ACT
# (0 = all relus on ACT). GPSIMD tensor_scalar measured 7.5us/op — unusable.
DVE_RELU_PERIOD = 0
# stage-1 rows with i % ACT_S1_MOD == ACT_S1_MOD//2 run on ACT to drain DVE
# (ACT's relu load already matches DVE's stage-1 load; keep stage-1 on DVE)
ACT_S1_MOD = 10 ** 9


def build_nc():
    # Bacc (not plain Bass): its compile() runs move_matmul_waits_to_ldweights
    # + generate_event_semaphores, needed to satisfy TRN2's 1-wait-per-matmul
    # hardware constraint.
    nc = bacc.Bacc("TRN2")

    # wpack = [w1b | w1a | w2dup | zbig] along free; bpack = [b1 | b2dup | b3]
    # zbig [128, 256]: cols 126,127 = [W3;0],[0;W3], zeros elsewhere. The
    # stage-3 stationary for slot k is the full-width 128-col window
    # zbig[:, 126-2k : 254-2k]: its W3 pair lands at array cols 2k,2k+1 and
    # every other column is zero, so each matmul writes the WHOLE bank
    # (slot 0 with start=True clears all has_written bits; later slots
    # accumulate +0 everywhere except their own 2 partitions).
    xT_d = nc.dram_tensor("xT", [D, N], BF16, kind="ExternalInput")
    xTi_d = nc.dram_tensor("xTi", [D, ROWS], BF16, kind="ExternalInput")
    wpack_d = nc.dram_tensor("wpack", [D, 3 * D + 256], BF16, kind="ExternalInput")
    bpack_d = nc.dram_tensor("bpack", [D, 3], F32, kind="ExternalInput")
    out_d = nc.dram_tensor("out", [ROWS, N], F32, kind="ExternalOutput")

    AT = mybir.ActivationFunctionType
    OP = mybir.AluOpType

    with tile.TileContext(nc) as tc:
        with tc.tile_pool(name="singles", bufs=1) as singles:
            xt = singles.tile([D, N], BF16)
            xti = singles.tile([D, ROWS], BF16)
            wpack = singles.tile([D, 3 * D + 256], BF16)
            bpack = singles.tile([D, 3], F32)
            hjsb = singles.tile([D, N], BF16)
            bias = singles.tile([D, ROWS], F32)

            # spread prep loads over several issue queues; gpsimd's SWDGE
            # queue comes up earliest after the NEFF preamble (and DVE is
            # idle during prep, so no shared-port contention yet)
            nc.gpsimd.dma_start(xt[:], xT_d[:])
            nc.scalar.dma_start(wpack[:], wpack_d[:])
            nc.sync.dma_start(xti[:], xTi_d[:])
            nc.sync.dma_start(bpack[:], bpack_d[:])
            w1b = wpack[:, 0:D]
            w1a = wpack[:, D:2 * D]
            w2d = wpack[:, 2 * D:3 * D]
            zbig = wpack[:, 3 * D:3 * D + 256]
            b1 = bpack[:, 0:1]
            b2 = bpack[:, 1:2]
            b3 = bpack[:, 2:3]

            # warm both ACT table sets (relu+sigmoid) under the DMA shadow
            warm = singles.tile([D, 1], F32)
            nc.vector.memset(warm[:], 0.0)
            nc.scalar.activation(warm[:], warm[:], AT.Relu)
            nc.scalar.activation(warm[:], warm[:], AT.Sigmoid)

            # ---- prep: hj (bf16) and per-row bias = hi + b1 (fp32) ----
            with tc.tile_pool(name="prep_ps", bufs=1, space="PSUM") as pps:
                hj_ps = pps.tile([D, N], F32)
                nc.tensor.matmul(hj_ps[:], lhsT=w1b[:], rhs=xt[:])
                # cast on ACT so it overlaps the bias add on DVE
                nc.scalar.activation(hjsb[:], hj_ps[:], AT.Copy)

                hi_ps = pps.tile([D, ROWS], F32)
                nc.tensor.matmul(hi_ps[:], lhsT=w1a[:], rhs=xti[:])
                nc.vector.tensor_scalar(
                    bias[:], hi_ps[:], b1[:, 0:1], None, OP.add
                )

            # ---- main loop: 2 pairs (4 rows) per q ----
            with (
                tc.tile_pool(name="spool", bufs=6) as spool,
                tc.tile_pool(name="h2pool", bufs=5) as h2pool,
                tc.tile_pool(name="opool", bufs=3) as opool,
                tc.tile_pool(name="ps2pool", bufs=3, space="PSUM") as ps2pool,
                tc.tile_pool(name="ps3pool", bufs=2, space="PSUM") as ps3pool,
            ):
                # software pipeline: stage-3 matmuls run LAG iterations behind
                # stage-2 so PE's FIFO never stalls waiting on the fresh relu.
                LAG = 2
                ps3 = None
                h2rs = {}
                for q in range(NQ + LAG):
                    if q < NQ:
                        rows = [4 * q + r for r in range(4)]
                        stile = spool.tile([D, 4 * N], BF16, tag="s")
                        ss = []
                        for r, i in enumerate(rows):
                            s = stile[:, r * N:(r + 1) * N]
                            if i % ACT_S1_MOD == ACT_S1_MOD // 2:
                                nc.scalar.activation(
                                    s, hjsb[:], AT.Relu,
                                    bias=bias[:, i:i + 1], scale=1.0,
                                )
                            else:
                                nc.vector.tensor_scalar(
                                    s, hjsb[:], bias[:, i:i + 1], 0.0,
                                    OP.add, OP.max,
                                )
                            ss.append(s)

                        ps2 = ps2pool.tile([D, 2 * N], F32)
                        nc.tensor.matmul(ps2[0:64, 0:N], lhsT=w2d[:, 0:64], rhs=ss[0])
                        nc.tensor.matmul(ps2[64:128, 0:N], lhsT=w2d[:, 64:128], rhs=ss[1])
                        nc.tensor.matmul(ps2[0:64, N:2 * N], lhsT=w2d[:, 0:64], rhs=ss[2])
                        nc.tensor.matmul(ps2[64:128, N:2 * N], lhsT=w2d[:, 64:128], rhs=ss[3])

                        h2r = h2pool.tile([D, 2 * N], BF16, tag="h2r")
                        if DVE_RELU_PERIOD and q % DVE_RELU_PERIOD == 0:
                            nc.vector.tensor_scalar(
                                h2r[:], ps2[:], b2[:, 0:1], 0.0, OP.add, OP.max
                            )
                        else:
                            nc.scalar.activation(
                                h2r[:], ps2[:], AT.Relu, bias=b2[:, 0:1], scale=1.0
                            )
                        h2rs[q] = h2r

                    if q >= LAG:
                        qq = q - LAG
                        h2r_l = h2rs.pop(qq)
                        # stage 3, dense-packed: 64 pairs accumulate into ONE
                        # psum bank. pair p: slot = p%64, strip s = slot//16,
                        # col u = slot%16 -> logits land at partition 32s+2u+r
                        # == out row (p*2+r) % 128. lhsT is a 32-col window of
                        # zwin whose leading zero cols add +0 to partitions
                        # already written (has_written accumulate semantics).
                        for a in range(2):
                            p = 2 * qq + a
                            slot = p % 64
                            if slot == 0:
                                ps3 = ps3pool.tile([D, N], F32)
                            nc.tensor.matmul(
                                ps3[:, :],
                                lhsT=zbig[:, 126 - 2 * slot:254 - 2 * slot],
                                rhs=h2r_l[:, N * a:N * a + N],
                                start=(slot == 0),
                                stop=(slot == 63),
                                skip_group_check=True,
                            )
                            if slot == 63:
                                g = p // 64  # 64 pairs = 128 contiguous rows
                                sig = opool.tile([D, N], F32, tag="sig")
                                nc.scalar.activation(
                                    sig[:], ps3[:], AT.Sigmoid,
                                    bias=b3[:, 0:1], scale=1.0,
                                )
                                nc.sync.dma_start(
                                    out_d[D * g:D * g + D, :], sig[:]
                                )
    nc.finalize()
    return nc


_CACHED_NC = None


def _get_nc():
    global _CACHED_NC
    if _CACHED_NC is None:
        _CACHED_NC = build_nc()
    return _CACHED_NC


def _host_prep(node_features, W1, b1, W2, b2, W3, b3):
    bf = ml_dtypes.bfloat16
    w1a = W1[:D]
    w1b = W1[D:]
    w2d = np.concatenate([W2, W2], axis=1)
    zbig = np.zeros((D, 256), np.float32)
    zbig[0:64, 126] = W3[:, 0]
    zbig[64:128, 127] = W3[:, 0]
    wpack = np.ascontiguousarray(
        np.concatenate([w1b, w1a, w2d, zbig], axis=1).astype(bf))
    bpack = np.ascontiguousarray(np.stack([
        b1, np.concatenate([b2, b2]), np.full(D, b3[0])
    ], axis=1).astype(np.float32))

    in_maps = []
    for c in range(NCORES):
        b, half = c // 2, c % 2
        xT = np.ascontiguousarray(node_features[b].T.astype(bf))
        xTi = np.ascontiguousarray(xT[:, half * ROWS:(half + 1) * ROWS])
        in_maps.append({
            "xT": xT, "xTi": xTi, "wpack": wpack, "bpack": bpack,
        })
    return in_maps


def run(node_features, W1, b1, W2, b2, W3, b3, **spmd_kwargs):
    """Run the bass kernel; returns (full_output, BassKernelResults)."""
    nc = _get_nc()
    in_maps = _host_prep(node_features, W1, b1, W2, b2, W3, b3)
    res = run_bass_kernel_spmd(nc, in_maps, core_ids=list(range(NCORES)), **spmd_kwargs)
    out = np.empty((B, N, N), np.float32)
    for c in range(NCORES):
        b, half = c // 2, c % 2
        out[b, half * ROWS:(half + 1) * ROWS, :] = res.results[c]["out"]
    return out, res


def kernel(node_features, W1, b1, W2, b2, W3, b3):
    out, _ = run(node_features, W1, b1, W2, b2, W3, b3)
    return out


# revision 47
# speedup vs baseline: 1.0023x; 1.0023x over previous
"""Trainium2 Bass kernel for the CoOccurrenceEncoder pairwise-MLP problem.

Reference computation (per batch b of 4, N=512 nodes, d=128):
    hi = x @ W1[:d]          # [N, d]
    hj = x @ W1[d:]          # [N, d]
    h  = relu(hi[:,None,:] + hj[None,:,:] + b1)       # [N, N, d]
    h2 = relu(h @ W2 + b2)                            # [N, N, 64]
    out = sigmoid((h2 @ W3 + b3)[..., 0])             # [N, N]

Sharding: 8 cores; core c handles batch c//2, i-rows [256*(c%2), 256*(c%2)+256).
Each core holds its batch's full hj [d=128 partitions, N=512] in SBUF (bf16) and
streams 256 i-rows; weights are tiny and replicated.

Per-core dataflow (d=128 on partitions everywhere):
  stage1 (DVE, bf16 2x_1P): S_i = relu(hj + (hi_i + b1)) via one dual-op
          tensor_scalar (add per-partition vector, max 0) per row -> SBUF bf16
  stage2 (PE): stationary [W2 | W2] (128x128 bf16); a row PAIR runs as two
          column-tiled matmuls that co-start -> h2 fp32 [128, 2*512] PSUM
  stage2b (ACT): relu(h2 + b2) PSUM -> SBUF bf16 (1x, PSUM-source bound)
  stage3 (PE, software-pipelined LAG iterations behind stage2): 64 pairs
          accumulate densely into ONE psum bank; pair slot k uses the
          128-wide stationary window zbig[:, 126-2k:254-2k] whose W3 pair
          sits at array cols 2k,2k+1 (all other cols zero -> accumulate +0),
          so logits for 128 consecutive i-rows land on partitions 0..127.
  stage4 (ACT): one sigmoid(logits + b3) [128,512] per 64 pairs -> one
          contiguous [128,512] HWDGE DMA to HBM.
"""

import numpy as np
import ml_dtypes

import concourse.bass as bass
import concourse.mybir as mybir
import concourse.tile as tile
from concourse import bacc
from concourse.bass_utils import run_bass_kernel_spmd

F32 = mybir.dt.float32
BF16 = mybir.dt.bfloat16

D = 128          # feature dim (= partitions)
N = 512          # nodes per batch
B = 4            # batches
NCORES = 8
ROWS = 256       # i-rows per core
PAIRS = ROWS // 2
NQ = PAIRS // 2  # loop iterations, 2 pairs (4 rows) each

# every DVE_RELU_PERIOD-th q does its stage2-relu on DVE instead of ACT
# (0 = all relus on ACT). GPSIMD tensor_scalar measured 7.5us/op — unusable.
DVE_RELU_PERIOD = 0
# stage-1 rows with i % ACT_S1_MOD == ACT_S1_MOD//2 run on ACT to drain DVE
# (ACT's relu load already matches DVE's stage-1 load; keep stage-1 on DVE)
ACT_S1_MOD = 10 ** 9


def build_nc():
    # Bacc (not plain Bass): its compile() runs move_matmul_waits_to_ldweights
    # + generate_event_semaphores, needed to satisfy TRN2's 1-wait-per-matmul
    # hardware constraint.
    nc = bacc.Bacc("TRN2")

    # wpack = [w1b | w1a | w2dup | zbig] along free; bpack = [b1 | b2dup | b3]
    # zbig [128, 256]: cols 126,127 = [W3;0],[0;W3], zeros elsewhere. The
    # stage-3 stationary for slot k is the full-width 128-col window
    # zbig[:, 126-2k : 254-2k]: its W3 pair lands at array cols 2k,2k+1 and
    # every other column is zero, so each matmul writes the WHOLE bank
    # (slot 0 with start=True clears all has_written bits; later slots
    # accumulate +0 everywhere except their own 2 partitions).
    xT_d = nc.dram_tensor("xT", [D, N], BF16, kind="ExternalInput")
    xTi_d = nc.dram_tensor("xTi", [D, ROWS], BF16, kind="ExternalInput")
    wpack_d = nc.dram_tensor("wpack", [D, 3 * D + 256], BF16, kind="ExternalInput")
    bpack_d = nc.dram_tensor("bpack", [D, 3], F32, kind="ExternalInput")
    out_d = nc.dram_tensor("out", [ROWS, N], F32, kind="ExternalOutput")

    AT = mybir.ActivationFunctionType
    OP = mybir.AluOpType

    with tile.TileContext(nc) as tc:
        with tc.tile_pool(name="singles", bufs=1) as singles:
            xt = singles.tile([D, N], BF16)
            xti = singles.tile([D, ROWS], BF16)
            wpack = singles.tile([D, 3 * D + 256], BF16)
            bpack = singles.tile([D, 3], F32)
            hjsb = singles.tile([D, N], BF16)
            bias = singles.tile([D, ROWS], F32)

            # spread prep loads over several issue queues; gpsimd's SWDGE
            # queue comes up earliest after the NEFF preamble (and DVE is
            # idle during prep, so no shared-port contention yet)
            nc.gpsimd.dma_start(xt[:], xT_d[:])
            nc.scalar.dma_start(wpack[:], wpack_d[:])
            nc.sync.dma_start(xti[:], xTi_d[:])
            nc.sync.dma_start(bpack[:], bpack_d[:])
            w1b = wpack[:, 0:D]
            w1a = wpack[:, D:2 * D]
            w2d = wpack[:, 2 * D:3 * D]
            zbig = wpack[:, 3 * D:3 * D + 256]
            b1 = bpack[:, 0:1]
            b2 = bpack[:, 1:2]
            b3 = bpack[:, 2:3]

            # warm both ACT table sets (relu+sigmoid) under the DMA shadow
            warm = singles.tile([D, 1], F32)
            nc.vector.memset(warm[:], 0.0)
            nc.scalar.activation(warm[:], warm[:], AT.Relu)
            nc.scalar.activation(warm[:], warm[:], AT.Sigmoid)

            # ---- prep: hj (bf16) and per-row bias = hi + b1 (fp32) ----
            with tc.tile_pool(name="prep_ps", bufs=1, space="PSUM") as pps:
                hj_ps = pps.tile([D, N], F32)
                nc.tensor.matmul(hj_ps[:], lhsT=w1b[:], rhs=xt[:])
                # cast on ACT so it overlaps the bias add on DVE
                nc.scalar.activation(hjsb[:], hj_ps[:], AT.Copy)

                hi_ps = pps.tile([D, ROWS], F32)
                nc.tensor.matmul(hi_ps[:], lhsT=w1a[:], rhs=xti[:])
                nc.vector.tensor_scalar(
                    bias[:], hi_ps[:], b1[:, 0:1], None, OP.add
                )

            # ---- main loop: 2 pairs (4 rows) per q ----
            with (
                tc.tile_pool(name="spool", bufs=8) as spool,
                tc.tile_pool(name="h2pool", bufs=6) as h2pool,
                tc.tile_pool(name="opool", bufs=3) as opool,
                tc.tile_pool(name="ps2pool", bufs=3, space="PSUM") as ps2pool,
                tc.tile_pool(name="ps3pool", bufs=2, space="PSUM") as ps3pool,
            ):
                # software pipeline: stage-3 matmuls run LAG iterations behind
                # stage-2 so PE's FIFO never stalls waiting on the fresh relu.
                LAG = 2
                ps3 = None
                h2rs = {}
                for q in range(NQ + LAG):
                    if q < NQ:
                        rows = [4 * q + r for r in range(4)]
                        stile = spool.tile([D, 4 * N], BF16, tag="s")
                        ss = []
                        for r, i in enumerate(rows):
                            s = stile[:, r * N:(r + 1) * N]
                            if i % ACT_S1_MOD == ACT_S1_MOD // 2:
                                nc.scalar.activation(
                                    s, hjsb[:], AT.Relu,
                                    bias=bias[:, i:i + 1], scale=1.0,
                                )
                            else:
                                nc.vector.tensor_scalar(
                                    s, hjsb[:], bias[:, i:i + 1], 0.0,
                                    OP.add, OP.max,
                                )
                            ss.append(s)

                        ps2 = ps2pool.tile([D, 2 * N], F32)
                        nc.tensor.matmul(ps2[0:64, 0:N], lhsT=w2d[:, 0:64], rhs=ss[0])
                        nc.tensor.matmul(ps2[64:128, 0:N], lhsT=w2d[:, 64:128], rhs=ss[1])
                        nc.tensor.matmul(ps2[0:64, N:2 * N], lhsT=w2d[:, 0:64], rhs=ss[2])
                        nc.tensor.matmul(ps2[64:128, N:2 * N], lhsT=w2d[:, 64:128], rhs=ss[3])

                        h2r = h2pool.tile([D, 2 * N], BF16, tag="h2r")
                        if DVE_RELU_PERIOD and q % DVE_RELU_PERIOD == 0:
                            nc.vector.tensor_scalar(
                                h2r[:], ps2[:], b2[:, 0:1], 0.0, OP.add, OP.max
                            )
                        else:
                            nc.scalar.activation(
                                h2r[:], ps2[:], AT.Relu, bias=b2[:, 0:1], scale=1.0
                            )
                        h2rs[q] = h2r

                    if q >= LAG:
                        qq = q - LAG
                        h2r_l = h2rs.pop(qq)
                        # stage 3, dense-packed: 64 pairs accumulate into ONE
                        # psum bank. pair p: slot = p%64, strip s = slot//16,
                        # col u = slot%16 -> logits land at partition 32s+2u+r
                        # == out row (p*2+r) % 128. lhsT is a 32-col window of
                        # zwin whose leading zero cols add +0 to partitions
                        # already written (has_written accumulate semantics).
                        for a in range(2):
                            p = 2 * qq + a
                            slot = p % 64
                            if slot == 0:
                                ps3 = ps3pool.tile([D, N], F32)
                            nc.tensor.matmul(
                                ps3[:, :],
                                lhsT=zbig[:, 126 - 2 * slot:254 - 2 * slot],
                                rhs=h2r_l[:, N * a:N * a + N],
                                start=(slot == 0),
                                stop=(slot == 63),
                                skip_group_check=True,
                            )
                            if slot == 63:
                                g = p // 64  # 64 pairs = 128 contiguous rows
                                sig = opool.tile([D, N], F32, tag="sig")
                                nc.scalar.activation(
                                    sig[:], ps3[:], AT.Sigmoid,
                                    bias=b3[:, 0:1], scale=1.0,
                                )
                                nc.sync.dma_start(
                                    out_d[D * g:D * g + D, :], sig[:]
                                )
    nc.finalize()
    return nc


_CACHED_NC = None


def _get_nc():
    global _CACHED_NC
    if _CACHED_NC is None:
        _CACHED_NC = build_nc()
    return _CACHED_NC


def _host_prep(node_features, W1, b1, W2, b2, W3, b3):
    bf = ml_dtypes.bfloat16
    w1a = W1[:D]
    w1b = W1[D:]
    w2d = np.concatenate([W2, W2], axis=1)
    zbig = np.zeros((D, 256), np.float32)
    zbig[0:64, 126] = W3[:, 0]
    zbig[64:128, 127] = W3[:, 0]
    wpack = np.ascontiguousarray(
        np.concatenate([w1b, w1a, w2d, zbig], axis=1).astype(bf))
    bpack = np.ascontiguousarray(np.stack([
        b1, np.concatenate([b2, b2]), np.full(D, b3[0])
    ], axis=1).astype(np.float32))

    in_maps = []
    for c in range(NCORES):
        b, half = c // 2, c % 2
        xT = np.ascontiguousarray(node_features[b].T.astype(bf))
        xTi = np.ascontiguousarray(xT[:, half * ROWS:(half + 1) * ROWS])
        in_maps.append({
            "xT": xT, "xTi": xTi, "wpack": wpack, "bpack": bpack,
        })
    return in_maps


def run(node_features, W1, b1, W2, b2, W3, b3, **spmd_kwargs):
    """Run the bass kernel; returns (full_output, BassKernelResults)."""
    nc = _get_nc()
    in_maps = _host_prep(node_features, W1, b1, W2, b2, W3, b3)
    res = run_bass_kernel_spmd(nc, in_maps, core_ids=list(range(NCORES)), **spmd_kwargs)
    out = np.empty((B, N, N), np.float32)
    for c in range(NCORES):
        b, half = c // 2, c % 2
        out[b, half * ROWS:(half + 1) * ROWS, :] = res.results[c]["out"]
    return out, res


def kernel(node_features, W1, b1, W2, b2, W3, b3):
    out, _ = run(node_features, W1, b1, W2, b2, W3, b3)
    return out
